# revision 2
# baseline (speedup 1.0000x reference)
"""LIF (leaky integrate-and-fire) forward kernel for Trainium2, 8-core SPMD.

Reference semantics (per element, scan over T):
    u = 0.5*u + x_t
    o_t = (u - 1 >= 0) ? 1.0 : 0.0
    u = u - o_t

Sharding: pure data parallel over batch B=32 -> 4 batches per core.
Per-core shard: x [4, 16, 128, 1024] f32; C=128 on the SBUF partition dim,
(b, h*w) on the free dim. Each timestep is three elementwise DVE ops on a
[128, 4096] tile:
    stt : u = (v * 0.5) + x_t          (scalar_tensor_tensor)
    ts  : o = (u >= 1.0)               (tensor_scalar is_ge)
    tt  : v = u - o                    (tensor_tensor subtract)
plus an ACT-engine copy casting o to uint8 for the store (exact for 0/1),
which quarters the output DMA traffic. All ops round exactly like the fp32
jax reference (0.5*v exact, one rounding add, exact subtract of 0/1), so
the output is bit-exact.

Raw bass (no TileContext): this walrus build caps embedded sync waits at 1
per DMA/DVE instruction, which Tile's sem assignment exceeds; standalone
wait_ge instructions have no such limit. DMA completion uses one semaphore
per buffer slot - slot reuse is serialized by the compute waits, so each
slot's DMAs complete in order even though the dynamic HW queues round-robin.
"""

import numpy as np

B, T, C, HW = 32, 16, 128, 1024
NCORES = 8
BLOC = B // NCORES  # 4 batches per core
FREE = BLOC * HW    # 4096
NX = 4              # x_t buffer slots
NF = 2              # o (f32) slots
NO = 2              # o (u8) slots

_cached = {}


def _build_nc():
    import concourse.bass as bass
    import concourse.mybir as mybir
    from contextlib import ExitStack

    f32 = mybir.dt.float32
    u8 = mybir.dt.uint8
    Alu = mybir.AluOpType

    nc = bass.Bass()
    x_d = nc.declare_dram_parameter("x", [BLOC, T, C, HW], f32, isOutput=False)
    o_d = nc.declare_dram_parameter("o", [BLOC, T, C, HW], u8, isOutput=True)

    with ExitStack() as ctx:
        v = ctx.enter_context(nc.sbuf_tensor("v", [C, FREE], f32))
        xt = [
            ctx.enter_context(nc.sbuf_tensor(f"xt{i}", [C, FREE], f32))
            for i in range(NX)
        ]
        of = [
            ctx.enter_context(nc.sbuf_tensor(f"of{i}", [C, FREE], f32))
            for i in range(NF)
        ]
        ot = [
            ctx.enter_context(nc.sbuf_tensor(f"ot{i}", [C, FREE], u8))
            for i in range(NO)
        ]
        s_x = [
            ctx.enter_context(nc.semaphore(f"s_x{i}")) for i in range(NX)
        ]
        s_out = [
            ctx.enter_context(nc.semaphore(f"s_out{i}")) for i in range(NO)
        ]
        s_dve = ctx.enter_context(nc.semaphore("s_dve"))
        s_act = ctx.enter_context(nc.semaphore("s_act"))
        block = ctx.enter_context(nc.Block())

        @block.sync
        def _(sync: bass.BassEngine):
            for t in range(T):
                if t >= NX:
                    # xt slot free once stt(t-NX) consumed it
                    sync.wait_ge(s_dve, 3 * (t - NX) + 1)
                sync.dma_start(
                    out=xt[t % NX][:, :].rearrange("p (b f) -> p b f", b=BLOC),
                    in_=x_d[:, t].rearrange("b c f -> c b f"),
                ).then_inc(s_x[t % NX], 16)
                if t >= 1:
                    sync.wait_ge(s_act, t)  # ot[t-1] written
                    sync.dma_start(
                        out=o_d[:, t - 1].rearrange("b c f -> c b f"),
                        in_=ot[(t - 1) % NO][:, :].rearrange(
                            "p (b f) -> p b f", b=BLOC
                        ),
                    ).then_inc(s_out[(t - 1) % NO], 16)
            sync.wait_ge(s_act, T)
            sync.dma_start(
                out=o_d[:, T - 1].rearrange("b c f -> c b f"),
                in_=ot[(T - 1) % NO][:, :].rearrange("p (b f) -> p b f", b=BLOC),
            ).then_inc(s_out[(T - 1) % NO], 16)
            # all output stores complete before kernel end
            for i in range(NO):
                n_i = (T - 1 - i) // NO + 1
                sync.wait_ge(s_out[i], 16 * n_i)

        @block.vector
        def _(vector: bass.BassEngine):
            vector.memset(v[:, :], 0.0)
            for t in range(T):
                vector.wait_ge(s_x[t % NX], 16 * (t // NX + 1))
                if t >= NF:
                    # of slot free once ACT(t-NF) copied it out
                    vector.wait_ge(s_act, t - NF + 1)
                vector.scalar_tensor_tensor(
                    out=v[:, :], in0=v[:, :], scalar=0.5, in1=xt[t % NX][:, :],
                    op0=Alu.mult, op1=Alu.add,
                ).then_inc(s_dve, 1)
                vector.tensor_scalar(
                    out=of[t % NF][:, :], in0=v[:, :], scalar1=1.0,
                    scalar2=None, op0=Alu.is_ge,
                ).then_inc(s_dve, 1)
                vector.tensor_tensor(
                    out=v[:, :], in0=v[:, :], in1=of[t % NF][:, :],
                    op=Alu.subtract,
                ).then_inc(s_dve, 1)

        @block.scalar
        def _(scalar: bass.BassEngine):
            for t in range(T):
                scalar.wait_ge(s_dve, 3 * t + 2)  # of[t] written
                if t >= NO:
                    # ot slot free once out-DMA(t-NO) completed
                    k = (t - NO - (t % NO)) // NO + 1
                    scalar.wait_ge(s_out[t % NO], 16 * k)
                scalar.copy(
                    out=ot[t % NO][:, :], in_=of[t % NF][:, :]
                ).then_inc(s_act, 1)

    return nc


def _get_nc():
    if "nc" not in _cached:
        _cached["nc"] = _build_nc()
    return _cached["nc"]


def kernel(x_seq: np.ndarray) -> np.ndarray:
    import os

    from concourse.bass_utils import run_bass_kernel_spmd

    x = np.ascontiguousarray(np.asarray(x_seq, dtype=np.float32)).reshape(
        B, T, C, HW
    )
    nc = _get_nc()
    in_maps = [{"x": x[i * BLOC : (i + 1) * BLOC]} for i in range(NCORES)]
    trace = bool(os.environ.get("LIF_TRACE"))
    out = run_bass_kernel_spmd(nc, in_maps, list(range(NCORES)), trace=trace)
    _cached["last_results"] = out
    o = np.concatenate([r["o"] for r in out.results], axis=0)
    return o.reshape(B, T, C, 32, 32).astype(np.float32)

